# revision 1
# baseline (speedup 1.0000x reference)
"""Multi-head attention (B=16, N=1024, C=768, H=12) on 8 TRN2 NeuronCores.

Sharding: data-parallel over batch — each core runs the full attention block
for 2 of the 16 batch elements; weights are replicated. No collectives.

Per-core Bass/Tile kernel (bf16 compute, fp32 accumulation):
  phase A: QKV projection. Q^T/K^T produced output-major ([d, token] per
    head-pair tile) by using w_qkv^T chunks as the stationary operand; V
    produced token-major and packed into "vp" tiles with a ones column per
    head ([v|1], 65 cols/head).
  phase B: per (head-pair, batch, n-half): S^T = K_chunk^T @ Q via row-tiled
    K=64 matmuls (both heads of a pair run in disjoint row-groups of the PE
    array), exp on ScalarE with the 1/sqrt(d) scale fused into the
    activation's input affine (no max-subtraction: |logits| < ~8 so fp32 exp
    is safe; softmax normalizes away the shift), then AV with the ones
    column appended so the softmax denominators fall out of the same matmul
    (PSUM row 64). Normalization: reciprocal of the sums row, a 1xK matmul
    broadcasts it across partitions, one multiply writes the normalized
    head-pair tile (odd head partition-shifted to rows 64:128).
  phase C: out = aT^T @ w_proj^T + bias at K=128 per head-pair.

The attention pipeline runs at n-half (512) granularity so the S^T PSUM
pool gets 4 slots — pipeline depth, not engine throughput, was the
bottleneck at coarser granularity.

Includes a workaround for this container's walrus: instructions may carry
at most ONE semaphore wait, so excess waits from the Tile scheduler are
hoisted onto injected EventSemaphore instructions in the BIR JSON.
"""

import json

import numpy as np
import ml_dtypes
from contextlib import ExitStack

import concourse.bass as bass
import concourse.tile as tile
import concourse.bass2jax as b2j
import concourse.bass_utils as bu
from concourse import mybir
from concourse.bass_utils import run_bass_kernel_spmd

F32 = mybir.dt.float32
BF16 = mybir.dt.bfloat16

DIM = 768
NH = 12
HD = 64
SCALE = HD ** -0.5
NB = 2          # batches per core
N = 1024        # tokens per batch
NT = NB * N
NCC = DIM // 128
NHP = NH // 2
SW = 65         # vp slot width per head
N_CORES = 8

# ---------------------------------------------------------------------------
# walrus single-wait workaround
# ---------------------------------------------------------------------------
_MAX_WAITS = 1
_orig_compile = bu.compile_bir_kernel


def _split_waits(bir_json: bytes) -> bytes:
    d = json.loads(bir_json)
    for f in d.get("functions", []):
        for blk in f.get("blocks", []):
            new_insts = []
            for inst in blk.get("instructions", []):
                si = inst.get("sync_info")
                waits = si.get("on_wait", []) if si else []
                if len(waits) > _MAX_WAITS:
                    extra, keep = waits[:-_MAX_WAITS], waits[-_MAX_WAITS:]
                    for ci in range(0, len(extra), _MAX_WAITS):
                        new_insts.append({
                            "debug": inst.get("debug", 0),
                            "engine": inst["engine"],
                            "ins": [],
                            "name": f"{inst['name']}-wsplit{ci}",
                            "opcode": "EventSemaphore",
                            "outs": [],
                            "sync_info": {
                                "on_update": [],
                                "on_wait": extra[ci:ci + _MAX_WAITS],
                            },
                        })
                    si["on_wait"] = keep
                new_insts.append(inst)
            blk["instructions"] = new_insts
    return json.dumps(d).encode()


def _patched_compile(bir_json, tmpdir, neff_name="file.neff"):
    return _orig_compile(_split_waits(bir_json), tmpdir, neff_name=neff_name)


def _install_patch():
    bu.compile_bir_kernel = _patched_compile
    b2j.compile_bir_kernel = _patched_compile


# ---------------------------------------------------------------------------
# per-core kernel builder
# ---------------------------------------------------------------------------
def build_attention_nc(reps: int = 1):
    nc = bass.Bass("TRN2", target_bir_lowering=False, debug=False)
    xT = nc.declare_dram_parameter("xT", [DIM, NT], BF16, isOutput=False)
    wqkvT = nc.declare_dram_parameter("wqkvT", [DIM, 3 * DIM], BF16, isOutput=False)
    wprojT = nc.declare_dram_parameter("wprojT", [DIM, DIM], BF16, isOutput=False)
    bias = nc.declare_dram_parameter("bias", [128, DIM], F32, isOutput=False)
    out = nc.declare_dram_parameter("out", [NT, DIM], F32, isOutput=True)

    with tile.TileContext(nc) as tc:
        for rep in range(reps):
            _emit(nc, tc, xT, wqkvT, wprojT, bias, out, rep)
    return nc


def _emit(nc, tc, xT, wqkvT, wprojT, bias, out, rep):
    R = f"r{rep}_"
    with ExitStack() as ctx:
        p_const = ctx.enter_context(tc.tile_pool(name=R + "const", bufs=1))
        p_w = ctx.enter_context(tc.tile_pool(name=R + "w", bufs=1))
        p_qk = ctx.enter_context(tc.tile_pool(name=R + "qk", bufs=1))
        p_vp = ctx.enter_context(tc.tile_pool(name=R + "vp", bufs=1))
        p_aT = ctx.enter_context(tc.tile_pool(name=R + "aT", bufs=1))

        bias_sb = p_const.tile([128, DIM], F32, name=R + "bias_sb")
        nc.sync.dma_start(bias_sb[:], bias[:])
        ones_sb = p_const.tile([128, 64], F32, name=R + "ones_sb")
        nc.vector.memset(ones_sb[:], 1.0)

        wq_t = []
        for c in range(NCC):
            t = p_w.tile([128, 3 * DIM], BF16, name=R + f"wq{c}")
            nc.sync.dma_start(t[:], wqkvT[c * 128:(c + 1) * 128, :])
            wq_t.append(t)
        wp_t = []
        for hp in range(NHP):
            t = p_w.tile([128, DIM], BF16, name=R + f"wp{hp}")
            nc.sync.dma_start(t[:], wprojT[hp * 128:(hp + 1) * 128, :])
            wp_t.append(t)
        xb = []
        for c in range(NCC):
            t = p_w.tile([128, NT], BF16, name=R + f"xb{c}")
            nc.sync.dma_start(t[:], xT[c * 128:(c + 1) * 128, :])
            xb.append(t)

        qT_t = [p_qk.tile([128, NT], BF16, name=R + f"qT{i}") for i in range(NHP)]
        kT_t = [p_qk.tile([128, NT], BF16, name=R + f"kT{i}") for i in range(NHP)]
        vp_t = [p_vp.tile([128, NH * SW], BF16, name=R + f"vp{i}")
                for i in range(NT // 128)]
        aT_t = {}

        # ---- phase A: V token-major + Q^T/K^T projections ----
        with tc.tile_pool(name=R + "psA1", bufs=4, space="PSUM") as p_ps1:
            for nn in range(NT // 128):
                psv = p_ps1.tile([128, 1024], F32, tag=R + "ps1",
                                 name=R + f"psv{nn}")
                for c in range(NCC):
                    nc.tensor.matmul(
                        psv[:, 0:512],
                        xb[c][:, nn * 128:(nn + 1) * 128],
                        wq_t[c][:, 2 * DIM:2 * DIM + 512],
                        start=(c == 0), stop=(c == NCC - 1),
                    )
                    nc.tensor.matmul(
                        psv[:, 512:768],
                        xb[c][:, nn * 128:(nn + 1) * 128],
                        wq_t[c][:, 2 * DIM + 512:3 * DIM],
                        start=(c == 0), stop=(c == NCC - 1),
                    )
                vp = vp_t[nn]
                nc.vector.memset(vp[:], 1.0)
                nc.vector.tensor_copy(
                    vp[:].rearrange("p (h e) -> p h e", e=SW)[:, :, 0:HD],
                    psv[:, 0:768].rearrange("p (h d) -> p h d", d=HD),
                )
            for hp in range(NHP):
                for which, dst in ((hp, qT_t[hp]), (NHP + hp, kT_t[hp])):
                    for half in range(2):
                        psq = p_ps1.tile([128, 1024], F32, tag=R + "ps1",
                                         name=R + f"psq{which}_{half}")
                        for c in range(NCC):
                            for nh in range(2):
                                nc.tensor.matmul(
                                    psq[:, nh * 512:(nh + 1) * 512],
                                    wq_t[c][:, which * 128:(which + 1) * 128],
                                    xb[c][:, half * 1024 + nh * 512:
                                          half * 1024 + (nh + 1) * 512],
                                    start=(c == 0), stop=(c == NCC - 1),
                                )
                        nc.vector.tensor_copy(
                            dst[:, half * 1024:(half + 1) * 1024], psq[:])

        # ---- phase B: attention, n-half granularity ----
        with tc.tile_pool(name=R + "psS", bufs=4, space="PSUM") as p_psS, \
             tc.tile_pool(name=R + "psAcc", bufs=2, space="PSUM") as p_psAcc, \
             tc.tile_pool(name=R + "E", bufs=8) as p_E, \
             tc.tile_pool(name=R + "rs", bufs=2) as p_rs, \
             tc.tile_pool(name=R + "bc", bufs=2) as p_bc:
            for hp in range(NHP):
                for b in range(NB):
                    boff = b * N
                    at = p_aT.tile([128, N], BF16, name=R + f"aT{b}_{hp}")
                    for nh in range(2):
                        noff = boff + nh * 512
                        pa = [p_psAcc.tile([128, 512], F32, tag=R + "psAcc",
                                           name=R + f"pa{b}_{hp}_{nh}_{i}")
                              for i in range(2)]
                        for mc in range(N // 128):
                            vslot = vp_t[(boff + mc * 128) // 128]
                            for hi in range(2):
                                h = 2 * hp + hi
                                pb_ = hi * 64
                                ps = p_psS.tile(
                                    [128, 512], F32, tag=R + "psS",
                                    name=R + f"psS{b}_{hp}_{nh}_{mc}_{hi}")
                                nc.tensor.matmul(
                                    ps[:],
                                    kT_t[hp][pb_:pb_ + 64,
                                             boff + mc * 128:boff + (mc + 1) * 128],
                                    qT_t[hp][pb_:pb_ + 64, noff:noff + 512],
                                    start=True, stop=True,
                                    tile_position=(pb_, 0),
                                )
                                e = p_E.tile([128, 512], BF16, tag=R + "E")
                                nc.scalar.activation(
                                    e[:], ps[:],
                                    mybir.ActivationFunctionType.Exp,
                                    scale=SCALE,
                                )
                                nc.tensor.matmul(
                                    pa[hi][0:65, :],
                                    vslot[:, h * SW:h * SW + SW],
                                    e[:],
                                    start=(mc == 0), stop=(mc == N // 128 - 1),
                                )
                        rs = p_rs.tile([128, 512], F32, tag=R + "rs")
                        nc.vector.reciprocal(rs[64:65, :], pa[0][64:65, :])
                        nc.vector.reciprocal(rs[96:97, :], pa[1][64:65, :])
                        psB = p_psS.tile([128, 512], F32, tag=R + "psS",
                                         name=R + f"psB{b}_{hp}_{nh}")
                        nc.tensor.matmul(
                            psB[0:64, :], ones_sb[64:65, 0:64], rs[64:65, :],
                            start=True, stop=True, tile_position=(64, 0),
                        )
                        nc.tensor.matmul(
                            psB[64:128, :], ones_sb[96:97, 0:64], rs[96:97, :],
                            start=True, stop=True, tile_position=(96, 64),
                        )
                        bc = p_bc.tile([128, 512], F32, tag=R + "bc")
                        nc.vector.tensor_copy(bc[:], psB[:])
                        nc.vector.tensor_mul(
                            at[0:64, nh * 512:(nh + 1) * 512],
                            pa[0][0:64, :], bc[0:64, :])
                        nc.vector.tensor_mul(
                            at[64:128, nh * 512:(nh + 1) * 512],
                            pa[1][0:64, :], bc[64:128, :])
                    aT_t[(b, hp)] = at

        # ---- phase C: output projection ----
        with tc.tile_pool(name=R + "psP", bufs=3, space="PSUM") as p_psP, \
             tc.tile_pool(name=R + "ob", bufs=4) as p_ob:
            for b in range(NB):
                for nn in range(N // 128):
                    pp = p_psP.tile([128, DIM], F32, tag=R + "psP",
                                    name=R + f"pp{b}_{nn}")
                    for hp in range(NHP):
                        lhs = aT_t[(b, hp)][:, nn * 128:(nn + 1) * 128]
                        nc.tensor.matmul(pp[:, 0:512], lhs, wp_t[hp][:, 0:512],
                                         start=(hp == 0), stop=(hp == NHP - 1))
                        nc.tensor.matmul(pp[:, 512:768], lhs,
                                         wp_t[hp][:, 512:768],
                                         start=(hp == 0), stop=(hp == NHP - 1))
                    ob = p_ob.tile([128, DIM], F32, tag=R + "ob")
                    nc.vector.tensor_add(ob[:], pp[:], bias_sb[:])
                    row0 = b * N + nn * 128
                    nc.sync.dma_start(out[row0:row0 + 128, :], ob[:])


# ---------------------------------------------------------------------------
# host wrapper
# ---------------------------------------------------------------------------
_CACHE = {}


def _prep_in_maps(x, w_qkv, w_proj, b_proj):
    x = np.asarray(x, dtype=np.float32)
    wqkvT = np.ascontiguousarray(np.asarray(w_qkv, dtype=np.float32).T
                                 ).astype(ml_dtypes.bfloat16)
    wprojT = np.ascontiguousarray(np.asarray(w_proj, dtype=np.float32).T
                                  ).astype(ml_dtypes.bfloat16)
    bias = np.broadcast_to(np.asarray(b_proj, dtype=np.float32),
                           (128, DIM)).copy()
    in_maps = []
    for c in range(N_CORES):
        xs = x[c * NB:(c + 1) * NB]                       # [2, 1024, 768]
        xT = np.ascontiguousarray(xs.transpose(2, 0, 1).reshape(DIM, NT))
        in_maps.append({
            "xT": xT.astype(ml_dtypes.bfloat16),
            "wqkvT": wqkvT,
            "wprojT": wprojT,
            "bias": bias,
        })
    return in_maps


def kernel(x, w_qkv, w_proj, b_proj):
    _install_patch()
    if "nc" not in _CACHE:
        _CACHE["nc"] = build_attention_nc(1)
    nc = _CACHE["nc"]
    in_maps = _prep_in_maps(x, w_qkv, w_proj, b_proj)
    res = run_bass_kernel_spmd(nc, in_maps, core_ids=list(range(N_CORES)))
    shards = [res.results[c]["out"].reshape(NB, N, DIM)
              for c in range(N_CORES)]
    return np.concatenate(shards, axis=0).astype(np.float32)
